# revision 30
# baseline (speedup 1.0000x reference)
"""CosVQ-EMA Trainium2 kernel (8 NeuronCores, bass/Tile).

Strategy (data-parallel over tokens, per sharding hint):
  - 16384 tokens sharded 2048/core; codebook [8192,128] replicated.
  - Per core, per 128-token tile: fp32 scores matmul (z_T stationary,
    normalized-codebook enT streamed) -> PSUM; ACT exp (1/|z| and the 1/T=10
    folded into the per-partition activation scale; softmax needs no max
    subtraction since |10*cos| <= 10) with accum_out giving row sums Z.
  - argmax via DVE max8 + max_index on E=exp(scores) (exp is monotone).
  - P_avg partials via M=1 PE matmuls (w=1/Z stationary, E streamed as
    bf16 made by an idle-GPSIMD copy) -> PSUM -> ACT copy -> DMA
    accum_op=add into a DRAM buffer.
  - counts+sums via per-tile merge matmul (duplicate-index groups summed via
    an is_equal selection matrix) + indirect DMA scatter with compute_op=add
    into a DRAM accumulator; within-tile duplicates redirected to garbage rows.
  - z_q via indirect DMA gather of emb rows.
  - One ReduceScatter over [8192,129] (sums|counts -> each core's 1024-code
    shard) + one small AllReduce over [66,128] (P_avg rows, commit and
    perplexity partials); each core then computes its EMA output shard and
    the (replicated) scalars.
Host side only shards inputs / concatenates outputs.
"""

import numpy as np

N = 16384
D = 128
K = 8192
NCORES = 8
NTOK = N // NCORES          # 2048 tokens per core
NT = NTOK // 128            # 16 tiles of 128 tokens
KSH = K // NCORES           # 1024 codes per core for EMA outputs
KC = K // 512               # 16 chunks of 512 codes
BETA = 0.25
DECAY = 0.8
INV_TEMP = 10.0
GARBAGE_ROWS = 256

_cached = {}


def _build():
    import concourse.bass as bass
    import concourse.mybir as mybir
    import concourse.tile as tile
    from concourse import bacc
    from concourse.masks import make_identity

    F32 = mybir.dt.float32
    BF16 = mybir.dt.bfloat16
    U32 = mybir.dt.uint32
    I32 = mybir.dt.int32
    AF = mybir.ActivationFunctionType
    ALU = mybir.AluOpType
    AX = mybir.AxisListType

    nc = bacc.Bacc("TRN2", target_bir_lowering=False, debug=False,
                   num_devices=NCORES)

    z_d = nc.dram_tensor("z", [NTOK, D], F32, kind="ExternalInput").ap()
    emb_d = nc.dram_tensor("emb", [K, D], F32, kind="ExternalInput").ap()
    cs_d = nc.dram_tensor("cs", [KSH], F32, kind="ExternalInput").ap()
    ema_d = nc.dram_tensor("ema", [KSH, D], F32, kind="ExternalInput").ap()

    zq_d = nc.dram_tensor("zq", [NTOK, D], F32, kind="ExternalOutput").ap()
    ncs_d = nc.dram_tensor("ncs", [KSH], F32, kind="ExternalOutput").ap()
    nema_d = nc.dram_tensor("nema", [KSH, D], F32, kind="ExternalOutput").ap()
    nemb_d = nc.dram_tensor("nemb", [KSH, D], F32, kind="ExternalOutput").ap()
    scal_d = nc.dram_tensor("scal", [4], F32, kind="ExternalOutput").ap()

    # DRAM accumulators. ar_sums rows: [0:8192] per-code [sums(128)|count];
    # rows [8192:] garbage rows absorbing within-tile duplicate scatters.
    ar_sums = nc.dram_tensor("ar_sums", [K + GARBAGE_ROWS, D + 1], F32).ap()
    ar_sums_o = nc.dram_tensor("ar_sums_o", [KSH, D + 1], F32).ap()
    # ar_pv rows [0:64] = P_avg sums as 64x128; row 64 col 0 = commit partial.
    ar_pv = nc.dram_tensor("ar_pv", [66, D], F32).ap()
    ar_pv_o = nc.dram_tensor("ar_pv_o", [66, D], F32, addr_space="Shared").ap()

    groups = [list(range(NCORES))]

    with tile.TileContext(nc) as tc:
        with tc.tile_pool(name="const", bufs=1) as cp, \
             tc.tile_pool(name="work", bufs=2) as wp, \
             tc.tile_pool(name="small", bufs=3) as sp, \
             tc.tile_pool(name="ebf", bufs=1) as ebp, \
             tc.tile_pool(name="ps_sc", bufs=2, space="PSUM") as ps_sc, \
             tc.tile_pool(name="ps_sm", bufs=2, space="PSUM") as ps_sm:

            ident = cp.tile([128, 128], F32)
            make_identity(nc, ident[:])
            ones_col = cp.tile([128, 1], F32)
            nc.vector.memset(ones_col[:], 1.0)
            zeros_row = cp.tile([128, D + 1], F32)
            nc.vector.memset(zeros_row[:], 0.0)
            iota_qp = cp.tile([128, 128], I32)
            nc.gpsimd.iota(iota_qp[:], pattern=[[1, 128]], base=0,
                           channel_multiplier=-1)
            iota_qp_f = cp.tile([128, 128], F32)
            nc.vector.tensor_copy(iota_qp_f[:], iota_qp[:])
            LT = cp.tile([128, 128], F32)
            nc.vector.tensor_scalar(LT[:], iota_qp_f[:], 0.0, None,
                                    op0=ALU.is_lt)
            gro_i = cp.tile([128, 1], I32)
            nc.gpsimd.iota(gro_i[:], pattern=[[0, 1]], base=K,
                           channel_multiplier=1)
            gro_f = cp.tile([128, 1], F32)
            nc.vector.tensor_copy(gro_f[:], gro_i[:])
            eps8 = cp.tile([128, 1], F32)
            nc.vector.memset(eps8[:], 1e-8)

            # ---- zero the DRAM accumulators ----
            arv = ar_sums.rearrange("(t p) x -> p t x", p=128)
            for t in range(arv.shape[1]):
                nc.sync.dma_start(out=arv[:, t, :], in_=zeros_row[:])
            nc.sync.dma_start(out=ar_pv, in_=zeros_row[:66, :D])

            # ---- codebook prep: enT = normalize(emb) transposed ----
            emb_sb = wp.tile([128, 64, 128], F32, tag="E")
            nc.sync.dma_start(out=emb_sb[:],
                              in_=emb_d.rearrange("(c p) d -> p c d", p=128))
            esq = wp.tile([128, 8192], F32, tag="E")
            nc.vector.tensor_tensor(out=esq[:].rearrange("p (c d) -> p c d", c=64),
                                    in0=emb_sb[:], in1=emb_sb[:], op=ALU.mult)
            en2 = cp.tile([128, 64], F32)
            nc.vector.reduce_sum(out=en2[:],
                                 in_=esq[:].rearrange("p (c d) -> p c d", c=64),
                                 axis=AX.X)
            enrm = cp.tile([128, 64], F32)
            nc.scalar.activation(out=enrm[:], in_=en2[:], func=AF.Sqrt)
            nc.vector.tensor_scalar(enrm[:], enrm[:], 1e-12, None, op0=ALU.max)
            erec = cp.tile([128, 64], F32)
            nc.vector.reciprocal(erec[:], enrm[:])
            for c in range(64):
                nc.vector.tensor_scalar(emb_sb[:, c, :], emb_sb[:, c, :],
                                        erec[:, c:c + 1], None, op0=ALU.mult)
            enTq = []
            for q in range(4):
                enT_part = cp.tile([128, 2048], F32, tag=f"enT{q}")
                enTq.append(enT_part)
            for c in range(64):
                pt = ps_sm.tile([128, 128], F32, tag="sm")
                nc.tensor.transpose(out=pt[:], in_=emb_sb[:, c, :],
                                    identity=ident[:])
                nc.scalar.copy(out=enTq[c // 16][:, (c % 16) * 128:
                                                 (c % 16 + 1) * 128],
                               in_=pt[:])

            # ---- z prep ----
            z_sb = cp.tile([128, NT, 128], F32)
            nc.sync.dma_start(out=z_sb[:],
                              in_=z_d.rearrange("(t p) d -> p t d", p=128))
            zsq = wp.tile([128, NTOK], F32, tag="E")
            nc.vector.tensor_tensor(out=zsq[:].rearrange("p (t d) -> p t d", t=NT),
                                    in0=z_sb[:], in1=z_sb[:], op=ALU.mult)
            zn2 = cp.tile([128, NT], F32)
            nc.vector.reduce_sum(out=zn2[:],
                                 in_=zsq[:].rearrange("p (t d) -> p t d", t=NT),
                                 axis=AX.X)
            znrm = cp.tile([128, NT], F32)
            nc.scalar.activation(out=znrm[:], in_=zn2[:], func=AF.Sqrt)
            nc.vector.tensor_scalar(znrm[:], znrm[:], 1e-12, None, op0=ALU.max)
            rnz10 = cp.tile([128, NT], F32)
            nc.vector.reciprocal(rnz10[:], znrm[:])
            nc.vector.tensor_scalar(rnz10[:], rnz10[:], INV_TEMP, None,
                                    op0=ALU.mult)
            zT = cp.tile([128, NT, 128], F32)
            for t in range(NT):
                pt = ps_sm.tile([128, 128], F32, tag="sm")
                nc.tensor.transpose(out=pt[:], in_=z_sb[:, t, :],
                                    identity=ident[:])
                nc.scalar.copy(out=zT[:, t, :], in_=pt[:])

            zq_all = cp.tile([128, NT, 128], F32)

            # ---- main loop over token tiles ----
            for t in range(NT):
                E = wp.tile([128, 8192], F32, tag="E")
                zparts = sp.tile([128, 8], F32, tag="zp")
                for g in range(8):
                    psc = ps_sc.tile([128, 1024], F32, tag="sc")
                    for j in range(2):
                        ch = 2 * g + j
                        nc.tensor.matmul(
                            out=psc[:, j * 512:(j + 1) * 512],
                            lhsT=zT[:, t, :],
                            rhs=enTq[ch // 4][:, (ch % 4) * 512:
                                              (ch % 4 + 1) * 512],
                            start=True, stop=True)
                    nc.scalar.activation(out=E[:, g * 1024:(g + 1) * 1024],
                                         in_=psc[:], func=AF.Exp,
                                         scale=rnz10[:, t:t + 1],
                                         accum_out=zparts[:, g:g + 1])
                Zrow = sp.tile([128, 1], F32, tag="zrow")
                nc.vector.reduce_sum(out=Zrow[:], in_=zparts[:], axis=AX.X)
                w = sp.tile([128, 1], F32, tag="w")
                nc.vector.reciprocal(w[:], Zrow[:])
                wc = sp.tile([128, 1], F32, tag="wc")
                nc.vector.tensor_scalar(wc[:], w[:], 1.00195694, None,
                                        op0=ALU.mult)
                w_bf = sp.tile([128, 1], BF16, tag="wbf")
                nc.vector.tensor_copy(w_bf[:], wc[:])
                # bf16 view of E: high half of each fp32 word (truncated bf16)
                E_bf = E[:].bitcast(BF16).rearrange("p (k two) -> p k two",
                                                    two=2)

                m8 = sp.tile([128, 8], F32, tag="m8")
                nc.vector.max(out=m8[:], in_=E[:])
                i8 = sp.tile([128, 8], U32, tag="i8")
                nc.vector.max_index(out=i8[:], in_max=m8[:], in_values=E[:])

                # P_avg partial: [1,512] = w^T @ E chunk (bf16), staged to a
                # [1,8192] SBUF row; one DMA accumulate into ar_pv per tile.
                pvst = ebp.tile([1, 8192], F32, tag="pvst")
                for g in range(8):
                    pv = ps_sm.tile([1, 1024], F32, tag="sm")
                    for j in range(2):
                        ch = 2 * g + j
                        nc.tensor.matmul(
                            out=pv[:, j * 512:(j + 1) * 512],
                            lhsT=w_bf[:],
                            rhs=E_bf[:, ch * 512:(ch + 1) * 512, 1],
                            start=True, stop=True)
                    nc.scalar.copy(out=pvst[:, g * 1024:(g + 1) * 1024],
                                   in_=pv[:])
                nc.gpsimd.dma_start(out=ar_pv[0:64, :], in_=pvst[:],
                                    accum_op=mybir.AluOpType.add)

                # ---- merge duplicate-index rows, dedup-redirect, scatter ----
                idx_f = sp.tile([128, 1], F32, tag="idxf")
                nc.vector.tensor_copy(idx_f[:], i8[:, 0:1])
                pT = ps_sm.tile([128, 128], F32, tag="sm")
                nc.tensor.transpose(out=pT[:],
                                    in_=idx_f[:].to_broadcast([128, 128]),
                                    identity=ident[:])
                idxT = sp.tile([128, 128], F32, tag="idxT")
                nc.vector.tensor_copy(idxT[:], pT[:])
                M = sp.tile([128, 128], F32, tag="M")
                nc.vector.tensor_tensor(out=M[:],
                                        in0=idx_f[:].to_broadcast([128, 128]),
                                        in1=idxT[:], op=ALU.is_equal)
                # duplicate mask: any earlier token with same idx
                MLT = sp.tile([128, 128], F32, tag="MLT")
                nc.vector.tensor_tensor(out=MLT[:], in0=M[:], in1=LT[:],
                                        op=ALU.mult)
                dup = sp.tile([128, 1], F32, tag="dup")
                nc.vector.reduce_sum(out=dup[:], in_=MLT[:], axis=AX.X)
                dmask = sp.tile([128, 1], U32, tag="dmask")
                nc.vector.tensor_scalar(dmask[:], dup[:], 0.0, None,
                                        op0=ALU.is_gt)
                idx_sc = sp.tile([128, 1], F32, tag="idxsc")
                nc.vector.tensor_copy(idx_sc[:], idx_f[:])
                nc.vector.copy_predicated(idx_sc[:], dmask[:], gro_f[:])
                idx_i = sp.tile([128, 1], I32, tag="idxi")
                nc.vector.tensor_copy(idx_i[:], idx_sc[:])

                payload = sp.tile([128, D + 1], F32, tag="pay")
                nc.vector.tensor_copy(payload[:, :D], z_sb[:, t, :])
                nc.vector.memset(payload[:, D:D + 1], 1.0)
                pm = ps_sm.tile([128, D + 1], F32, tag="sm")
                nc.tensor.matmul(out=pm[:], lhsT=M[:], rhs=payload[:],
                                 start=True, stop=True)
                merged = sp.tile([128, D + 1], F32, tag="mrg")
                nc.vector.tensor_copy(merged[:], pm[:])
                nc.gpsimd.indirect_dma_start(
                    out=ar_sums,
                    out_offset=bass.IndirectOffsetOnAxis(ap=idx_i[:], axis=0),
                    in_=merged[:], in_offset=None,
                    compute_op=ALU.add)

                # z_q gather
                nc.gpsimd.indirect_dma_start(
                    out=zq_all[:, t, :], out_offset=None, in_=emb_d,
                    in_offset=bass.IndirectOffsetOnAxis(ap=i8[:, 0:1], axis=0))

            # ---- epilogue: z_q_ste + commit partial ----
            dif = wp.tile([128, NTOK], F32, tag="E")
            zq_flat = zq_all[:].rearrange("p t d -> p (t d)")
            z_flat = z_sb[:].rearrange("p t d -> p (t d)")
            nc.vector.tensor_tensor(out=dif[:], in0=zq_flat, in1=z_flat,
                                    op=ALU.subtract)
            ste = wp.tile([128, NTOK], F32, tag="E")
            nc.vector.tensor_tensor(out=ste[:], in0=z_flat, in1=dif[:],
                                    op=ALU.add)
            nc.sync.dma_start(
                out=zq_d.rearrange("(t p) d -> p t d", p=128),
                in_=ste[:].rearrange("p (t d) -> p t d", t=NT))
            csum = sp.tile([128, 1], F32, tag="csum")
            nc.scalar.activation(out=dif[:], in_=dif[:], func=AF.Square,
                                 accum_out=csum[:])
            pc1 = ps_sm.tile([1, 1], F32, tag="sm")
            nc.tensor.matmul(out=pc1[:], lhsT=csum[:], rhs=ones_col[:],
                             start=True, stop=True)
            cstage = sp.tile([1, 1], F32, tag="cst")
            nc.scalar.activation(out=cstage[:], in_=pc1[:], func=AF.Copy,
                                 scale=BETA / (N * D))
            nc.gpsimd.dma_start(out=ar_pv[64:65, 0:1], in_=cstage[:],
                              accum_op=mybir.AluOpType.add)

            # ---- collectives ----
            nc.gpsimd.collective_compute(
                "ReduceScatter", mybir.AluOpType.add, ins=[ar_sums[0:K, :]],
                outs=[ar_sums_o[:]], replica_groups=groups)

            # ---- post-AR: EMA outputs for this core's 1024-code shard ----
            shard = cp.tile([128, 8, D + 1], F32)
            nc.sync.dma_start(
                out=shard[:],
                in_=ar_sums_o[:].rearrange("(c p) x -> p c x", p=128))
            cssb = cp.tile([128, 8], F32)
            nc.sync.dma_start(out=cssb[:],
                              in_=cs_d.rearrange("(c p) -> p c", p=128))
            emasb = cp.tile([128, 8, D], F32)
            nc.sync.dma_start(out=emasb[:],
                              in_=ema_d.rearrange("(c p) d -> p c d", p=128))

            ncs_sb = cp.tile([128, 8], F32)
            cnt_v = shard[:, :, D:D + 1].rearrange("p c o -> p (c o)")
            nc.vector.tensor_scalar(ncs_sb[:], cnt_v, 1 - DECAY, None,
                                    op0=ALU.mult)
            cs_sc = sp.tile([128, 8], F32, tag="cssc")
            nc.vector.tensor_scalar(cs_sc[:], cssb[:], DECAY, None,
                                    op0=ALU.mult)
            nc.vector.tensor_tensor(out=ncs_sb[:], in0=ncs_sb[:], in1=cs_sc[:],
                                    op=ALU.add)
            nc.sync.dma_start(out=ncs_d.rearrange("(c p) -> p c", p=128),
                              in_=ncs_sb[:])

            nema_sb = cp.tile([128, 8, D], F32)
            nc.vector.tensor_scalar(nema_sb[:], shard[:, :, :D],
                                    1 - DECAY, None, op0=ALU.mult)
            ema_sc = cp.tile([128, 8, D], F32)
            nc.vector.tensor_scalar(ema_sc[:], emasb[:],
                                    DECAY, None, op0=ALU.mult)
            nc.vector.tensor_tensor(out=nema_sb[:], in0=nema_sb[:],
                                    in1=ema_sc[:], op=ALU.add)
            nc.sync.dma_start(out=nema_d.rearrange("(c p) d -> p c d", p=128),
                              in_=nema_sb[:])

            den = sp.tile([128, 8], F32, tag="den")
            nc.vector.tensor_scalar(den[:], ncs_sb[:], 1e-5, None, op0=ALU.max)
            rden = sp.tile([128, 8], F32, tag="rden")
            nc.vector.reciprocal(rden[:], den[:])
            nemb_sb = cp.tile([128, 8, D], F32)
            for c in range(8):
                nc.vector.tensor_scalar(nemb_sb[:, c, :], nema_sb[:, c, :],
                                        rden[:, c:c + 1], None, op0=ALU.mult)
            nc.sync.dma_start(out=nemb_d.rearrange("(c p) d -> p c d", p=128),
                              in_=nemb_sb[:])

            # ---- perplexity partial over this core's shard ----
            em = sp.tile([128, 8], F32, tag="em")
            cnt_g = shard[:, :, D:D + 1].rearrange("p c o -> p (c o)")
            nc.vector.tensor_scalar(em[:], cnt_g, 1.0 / N, None, op0=ALU.mult)
            lg = sp.tile([128, 8], F32, tag="lg")
            nc.scalar.activation(out=lg[:], in_=em[:], func=AF.Ln,
                                 bias=eps8[:])
            nc.vector.tensor_tensor(out=lg[:], in0=lg[:], in1=em[:],
                                    op=ALU.mult)
            ppart = sp.tile([128, 1], F32, tag="ppart")
            nc.vector.reduce_sum(out=ppart[:], in_=lg[:], axis=AX.X)
            pp1 = ps_sm.tile([1, 1], F32, tag="sm")
            nc.tensor.matmul(out=pp1[:], lhsT=ppart[:], rhs=ones_col[:],
                             start=True, stop=True)
            pstage = sp.tile([1, 1], F32, tag="pst")
            nc.scalar.copy(out=pstage[:], in_=pp1[:])
            nc.gpsimd.dma_start(out=ar_pv[65:66, 0:1], in_=pstage[:],
                                accum_op=mybir.AluOpType.add)
            # second collective: P_avg rows + commit + perplexity partials
            nc.gpsimd.collective_compute(
                "AllReduce", mybir.AluOpType.add, ins=[ar_pv[:]],
                outs=[ar_pv_o[:]], replica_groups=groups)
            perpsum = sp.tile([1, 1], F32, tag="perp0")
            nc.sync.dma_start(out=perpsum[:], in_=ar_pv_o[65:66, 0:1])
            perp = sp.tile([1, 1], F32, tag="perp")
            nc.scalar.activation(out=perp[:], in_=perpsum[:], func=AF.Exp,
                                 scale=-1.0)

            # entropy from P_avg
            pvsb = cp.tile([64, D], F32, tag="pvsb")
            nc.sync.dma_start(out=pvsb[:], in_=ar_pv_o[0:64, :])
            Pm = sp.tile([64, D], F32, tag="Pm")
            nc.vector.tensor_scalar(Pm[:], pvsb[:], 1.0 / N, 1e-8,
                                    op0=ALU.mult, op1=ALU.add)
            lP = sp.tile([64, D], F32, tag="lP")
            nc.scalar.activation(out=lP[:], in_=Pm[:], func=AF.Ln, bias=0.0)
            nc.vector.tensor_tensor(out=lP[:], in0=lP[:], in1=Pm[:],
                                    op=ALU.mult)
            ered = sp.tile([64, 1], F32, tag="ered")
            nc.vector.reduce_sum(out=ered[:], in_=lP[:], axis=AX.X)
            pe1 = ps_sm.tile([1, 1], F32, tag="sm")
            nc.tensor.matmul(out=pe1[:], lhsT=ered[:], rhs=ones_col[:64, :],
                             start=True, stop=True)
            ent = sp.tile([1, 1], F32, tag="ent")
            nc.scalar.activation(out=ent[:], in_=pe1[:], func=AF.Copy,
                                 scale=-1.0)

            commit_sb = sp.tile([1, 1], F32, tag="comm")
            nc.sync.dma_start(out=commit_sb[:], in_=ar_pv_o[64:65, 0:1])

            scal_sb = sp.tile([1, 4], F32, tag="scal")
            nc.vector.tensor_copy(scal_sb[:, 0:1], commit_sb[:])
            nc.vector.tensor_copy(scal_sb[:, 1:2], perp[:])
            nc.vector.tensor_copy(scal_sb[:, 2:3], ent[:])
            nc.vector.memset(scal_sb[:, 3:4], 0.0)
            nc.sync.dma_start(out=scal_d[None, :], in_=scal_sb[:])

    nc.compile()
    return nc


def _get_nc():
    if "nc" not in _cached:
        _cached["nc"] = _build()
    return _cached["nc"]


def kernel(z, emb_weight, cluster_size, ema_embedding_data):
    from concourse.bass_utils import run_bass_kernel_spmd

    z = np.ascontiguousarray(z, dtype=np.float32)
    emb_weight = np.ascontiguousarray(emb_weight, dtype=np.float32)
    cluster_size = np.ascontiguousarray(cluster_size, dtype=np.float32)
    ema = np.ascontiguousarray(ema_embedding_data, dtype=np.float32)

    zf = z.reshape(N, D)
    in_maps = []
    for c in range(NCORES):
        in_maps.append({
            "z": zf[c * NTOK:(c + 1) * NTOK],
            "emb": emb_weight,
            "cs": cluster_size[c * KSH:(c + 1) * KSH],
            "ema": ema[c * KSH:(c + 1) * KSH],
        })
    nc = _get_nc()
    res = run_bass_kernel_spmd(nc, in_maps, core_ids=list(range(NCORES)))
    rs = res.results

    z_q_ste = np.concatenate([rs[c]["zq"] for c in range(NCORES)], axis=0)
    z_q_ste = z_q_ste.reshape(z.shape)
    new_cluster = np.concatenate([rs[c]["ncs"] for c in range(NCORES)])
    new_ema = np.concatenate([rs[c]["nema"] for c in range(NCORES)], axis=0)
    new_emb = np.concatenate([rs[c]["nemb"] for c in range(NCORES)], axis=0)
    scal = rs[0]["scal"]
    commit = np.float32(scal[0])
    perp = np.float32(scal[1])
    ent = np.float32(scal[2])
    return (z_q_ste, commit, perp, ent, new_cluster, new_ema, new_emb)


# revision 31
# speedup vs baseline: 1.1855x; 1.1855x over previous
"""CosVQ-EMA Trainium2 kernel (8 NeuronCores, bass/Tile).

Strategy (data-parallel over tokens, per sharding hint):
  - 16384 tokens sharded 2048/core; codebook [8192,128] replicated.
  - Per core, per 128-token tile: fp32 scores matmul (z_T stationary,
    normalized-codebook enT streamed) -> PSUM; ACT exp (1/|z| and the 1/T=10
    folded into the per-partition activation scale; softmax needs no max
    subtraction since |10*cos| <= 10) with accum_out giving row sums Z.
  - argmax via DVE max8 + max_index on E=exp(scores) (exp is monotone).
  - P_avg partials via M=1 PE matmuls (w=1/Z stationary, E streamed as
    bf16 made by an idle-GPSIMD copy) -> PSUM -> ACT copy -> DMA
    accum_op=add into a DRAM buffer.
  - counts+sums via per-tile merge matmul (duplicate-index groups summed via
    an is_equal selection matrix) + indirect DMA scatter with compute_op=add
    into a DRAM accumulator; within-tile duplicates redirected to garbage rows.
  - z_q via indirect DMA gather of emb rows.
  - One ReduceScatter over [8192,129] (sums|counts -> each core's 1024-code
    shard) + one small AllReduce over [66,128] (P_avg rows, commit and
    perplexity partials); each core then computes its EMA output shard and
    the (replicated) scalars.
Host side only shards inputs / concatenates outputs.
"""

import numpy as np

N = 16384
D = 128
K = 8192
NCORES = 8
NTOK = N // NCORES          # 2048 tokens per core
NT = NTOK // 128            # 16 tiles of 128 tokens
KSH = K // NCORES           # 1024 codes per core for EMA outputs
KC = K // 512               # 16 chunks of 512 codes
BETA = 0.25
DECAY = 0.8
INV_TEMP = 10.0
GARBAGE_ROWS = 256

_cached = {}


def _build():
    import concourse.bass as bass
    import concourse.mybir as mybir
    import concourse.tile as tile
    from concourse import bacc
    from concourse.masks import make_identity

    F32 = mybir.dt.float32
    BF16 = mybir.dt.bfloat16
    U32 = mybir.dt.uint32
    I32 = mybir.dt.int32
    AF = mybir.ActivationFunctionType
    ALU = mybir.AluOpType
    AX = mybir.AxisListType

    nc = bacc.Bacc("TRN2", target_bir_lowering=False, debug=False,
                   num_devices=NCORES)

    z_d = nc.dram_tensor("z", [NTOK, D], F32, kind="ExternalInput").ap()
    emb_d = nc.dram_tensor("emb", [K, D], F32, kind="ExternalInput").ap()
    cs_d = nc.dram_tensor("cs", [KSH], F32, kind="ExternalInput").ap()
    ema_d = nc.dram_tensor("ema", [KSH, D], F32, kind="ExternalInput").ap()

    zq_d = nc.dram_tensor("zq", [NTOK, D], F32, kind="ExternalOutput").ap()
    ncs_d = nc.dram_tensor("ncs", [KSH], F32, kind="ExternalOutput").ap()
    nema_d = nc.dram_tensor("nema", [KSH, D], F32, kind="ExternalOutput").ap()
    nemb_d = nc.dram_tensor("nemb", [KSH, D], F32, kind="ExternalOutput").ap()
    scal_d = nc.dram_tensor("scal", [4], F32, kind="ExternalOutput").ap()

    # DRAM accumulators. ar_sums rows: [0:8192] per-code [sums(128)|count];
    # rows [8192:] garbage rows absorbing within-tile duplicate scatters.
    ar_sums = nc.dram_tensor("ar_sums", [K + GARBAGE_ROWS, D + 1], F32).ap()
    ar_sums_o = nc.dram_tensor("ar_sums_o", [KSH, D + 1], F32).ap()
    # ar_pv rows [0:64] = P_avg sums as 64x128; row 64 col 0 = commit partial.
    ar_pv = nc.dram_tensor("ar_pv", [66, D], F32).ap()
    ar_pv_o = nc.dram_tensor("ar_pv_o", [66, D], F32, addr_space="Shared").ap()

    groups = [list(range(NCORES))]

    with tile.TileContext(nc) as tc:
        with tc.tile_pool(name="const", bufs=1) as cp, \
             tc.tile_pool(name="work", bufs=2) as wp, \
             tc.tile_pool(name="small", bufs=3) as sp, \
             tc.tile_pool(name="ebf", bufs=1) as ebp, \
             tc.tile_pool(name="ps_sc", bufs=2, space="PSUM") as ps_sc, \
             tc.tile_pool(name="ps_sm", bufs=2, space="PSUM") as ps_sm:

            ident = cp.tile([128, 128], F32)
            make_identity(nc, ident[:])
            ones_col = cp.tile([128, 1], F32)
            nc.vector.memset(ones_col[:], 1.0)
            zeros_row = cp.tile([128, D + 1], F32)
            nc.vector.memset(zeros_row[:], 0.0)
            iota_qp = cp.tile([128, 128], I32)
            nc.gpsimd.iota(iota_qp[:], pattern=[[1, 128]], base=0,
                           channel_multiplier=-1)
            iota_qp_f = cp.tile([128, 128], F32)
            nc.vector.tensor_copy(iota_qp_f[:], iota_qp[:])
            LT = cp.tile([128, 128], F32)
            nc.vector.tensor_scalar(LT[:], iota_qp_f[:], 0.0, None,
                                    op0=ALU.is_lt)
            gro_i = cp.tile([128, 1], I32)
            nc.gpsimd.iota(gro_i[:], pattern=[[0, 1]], base=K,
                           channel_multiplier=1)
            gro_f = cp.tile([128, 1], F32)
            nc.vector.tensor_copy(gro_f[:], gro_i[:])
            eps8 = cp.tile([128, 1], F32)
            nc.vector.memset(eps8[:], 1e-8)

            # ---- zero the DRAM accumulators ----
            arv = ar_sums.rearrange("(t p) x -> p t x", p=128)
            for t in range(arv.shape[1]):
                nc.sync.dma_start(out=arv[:, t, :], in_=zeros_row[:])
            nc.sync.dma_start(out=ar_pv, in_=zeros_row[:66, :D])

            # ---- codebook prep: enT = normalize(emb) transposed ----
            emb_sb = wp.tile([128, 64, 128], F32, tag="E")
            nc.sync.dma_start(out=emb_sb[:],
                              in_=emb_d.rearrange("(c p) d -> p c d", p=128))
            esq = wp.tile([128, 8192], F32, tag="E")
            nc.vector.tensor_tensor(out=esq[:].rearrange("p (c d) -> p c d", c=64),
                                    in0=emb_sb[:], in1=emb_sb[:], op=ALU.mult)
            en2 = cp.tile([128, 64], F32)
            nc.vector.reduce_sum(out=en2[:],
                                 in_=esq[:].rearrange("p (c d) -> p c d", c=64),
                                 axis=AX.X)
            enrm = cp.tile([128, 64], F32)
            nc.scalar.activation(out=enrm[:], in_=en2[:], func=AF.Sqrt)
            nc.vector.tensor_scalar(enrm[:], enrm[:], 1e-12, None, op0=ALU.max)
            erec = cp.tile([128, 64], F32)
            nc.vector.reciprocal(erec[:], enrm[:])
            for c in range(64):
                nc.vector.tensor_scalar(emb_sb[:, c, :], emb_sb[:, c, :],
                                        erec[:, c:c + 1], None, op0=ALU.mult)
            enTq = []
            for q in range(4):
                enT_part = cp.tile([128, 2048], F32, tag=f"enT{q}")
                enTq.append(enT_part)
            for c in range(64):
                pt = ps_sm.tile([128, 128], F32, tag="sm")
                nc.tensor.transpose(out=pt[:], in_=emb_sb[:, c, :],
                                    identity=ident[:])
                nc.scalar.copy(out=enTq[c // 16][:, (c % 16) * 128:
                                                 (c % 16 + 1) * 128],
                               in_=pt[:])

            # ---- z prep ----
            z_sb = cp.tile([128, NT, 128], F32)
            nc.sync.dma_start(out=z_sb[:],
                              in_=z_d.rearrange("(t p) d -> p t d", p=128))
            zsq = wp.tile([128, NTOK], F32, tag="E")
            nc.vector.tensor_tensor(out=zsq[:].rearrange("p (t d) -> p t d", t=NT),
                                    in0=z_sb[:], in1=z_sb[:], op=ALU.mult)
            zn2 = cp.tile([128, NT], F32)
            nc.vector.reduce_sum(out=zn2[:],
                                 in_=zsq[:].rearrange("p (t d) -> p t d", t=NT),
                                 axis=AX.X)
            znrm = cp.tile([128, NT], F32)
            nc.scalar.activation(out=znrm[:], in_=zn2[:], func=AF.Sqrt)
            nc.vector.tensor_scalar(znrm[:], znrm[:], 1e-12, None, op0=ALU.max)
            rnz10 = cp.tile([128, NT], F32)
            nc.vector.reciprocal(rnz10[:], znrm[:])
            nc.vector.tensor_scalar(rnz10[:], rnz10[:], INV_TEMP, None,
                                    op0=ALU.mult)
            zT = cp.tile([128, NT, 128], F32)
            for t in range(NT):
                pt = ps_sm.tile([128, 128], F32, tag="sm")
                nc.tensor.transpose(out=pt[:], in_=z_sb[:, t, :],
                                    identity=ident[:])
                nc.scalar.copy(out=zT[:, t, :], in_=pt[:])

            zq_all = cp.tile([128, NT, 128], F32)

            # ---- main loop over token tiles ----
            for t in range(NT):
                E = wp.tile([128, 8192], F32, tag="E")
                zparts = sp.tile([128, 8], F32, tag="zp")
                for g in range(8):
                    psc = ps_sc.tile([128, 1024], F32, tag="sc")
                    for j in range(2):
                        ch = 2 * g + j
                        nc.tensor.matmul(
                            out=psc[:, j * 512:(j + 1) * 512],
                            lhsT=zT[:, t, :],
                            rhs=enTq[ch // 4][:, (ch % 4) * 512:
                                              (ch % 4 + 1) * 512],
                            start=True, stop=True)
                    nc.scalar.activation(out=E[:, g * 1024:(g + 1) * 1024],
                                         in_=psc[:], func=AF.Exp,
                                         scale=rnz10[:, t:t + 1],
                                         accum_out=zparts[:, g:g + 1])
                Zrow = sp.tile([128, 1], F32, tag="zrow")
                nc.vector.reduce_sum(out=Zrow[:], in_=zparts[:], axis=AX.X)
                w = sp.tile([128, 1], F32, tag="w")
                nc.vector.reciprocal(w[:], Zrow[:])
                wc = sp.tile([128, 1], F32, tag="wc")
                nc.vector.tensor_scalar(wc[:], w[:], 1.00195694, None,
                                        op0=ALU.mult)
                w_bf = sp.tile([128, 1], BF16, tag="wbf")
                nc.vector.tensor_copy(w_bf[:], wc[:])
                # bf16 view of E: high half of each fp32 word (truncated bf16)
                E_bf = E[:].bitcast(BF16).rearrange("p (k two) -> p k two",
                                                    two=2)

                m16 = sp.tile([128, 16], F32, tag="m16")
                nc.vector.max(out=m16[:, 0:8], in_=E[:, 0:4096])
                nc.vector.max(out=m16[:, 8:16], in_=E[:, 4096:8192])
                m8 = sp.tile([128, 8], F32, tag="m8")
                nc.vector.max(out=m8[:], in_=m16[:])
                i8 = sp.tile([128, 8], U32, tag="i8")
                nc.vector.max_index(out=i8[:], in_max=m8[:], in_values=E[:])

                # P_avg partial: [1,512] = w^T @ E chunk (bf16), staged to a
                # [1,8192] SBUF row; one DMA accumulate into ar_pv per tile.
                pvst = ebp.tile([1, 8192], F32, tag="pvst")
                for g in range(8):
                    pv = ps_sm.tile([1, 1024], F32, tag="sm")
                    for j in range(2):
                        ch = 2 * g + j
                        nc.tensor.matmul(
                            out=pv[:, j * 512:(j + 1) * 512],
                            lhsT=w_bf[:],
                            rhs=E_bf[:, ch * 512:(ch + 1) * 512, 1],
                            start=True, stop=True)
                    nc.scalar.copy(out=pvst[:, g * 1024:(g + 1) * 1024],
                                   in_=pv[:])
                nc.gpsimd.dma_start(out=ar_pv[0:64, :], in_=pvst[:],
                                    accum_op=mybir.AluOpType.add)

                # ---- merge duplicate-index rows, dedup-redirect, scatter ----
                idx_f = sp.tile([128, 1], F32, tag="idxf")
                nc.vector.tensor_copy(idx_f[:], i8[:, 0:1])
                pT = ps_sm.tile([128, 128], F32, tag="sm")
                nc.tensor.transpose(out=pT[:],
                                    in_=idx_f[:].to_broadcast([128, 128]),
                                    identity=ident[:])
                idxT = sp.tile([128, 128], F32, tag="idxT")
                nc.vector.tensor_copy(idxT[:], pT[:])
                M = sp.tile([128, 128], F32, tag="M")
                nc.vector.tensor_tensor(out=M[:],
                                        in0=idx_f[:].to_broadcast([128, 128]),
                                        in1=idxT[:], op=ALU.is_equal)
                # duplicate mask: any earlier token with same idx
                MLT = sp.tile([128, 128], F32, tag="MLT")
                nc.vector.tensor_tensor(out=MLT[:], in0=M[:], in1=LT[:],
                                        op=ALU.mult)
                dup = sp.tile([128, 1], F32, tag="dup")
                nc.vector.reduce_sum(out=dup[:], in_=MLT[:], axis=AX.X)
                dmask = sp.tile([128, 1], U32, tag="dmask")
                nc.vector.tensor_scalar(dmask[:], dup[:], 0.0, None,
                                        op0=ALU.is_gt)
                idx_sc = sp.tile([128, 1], F32, tag="idxsc")
                nc.vector.tensor_copy(idx_sc[:], idx_f[:])
                nc.vector.copy_predicated(idx_sc[:], dmask[:], gro_f[:])
                idx_i = sp.tile([128, 1], I32, tag="idxi")
                nc.vector.tensor_copy(idx_i[:], idx_sc[:])

                payload = sp.tile([128, D + 1], F32, tag="pay")
                nc.vector.tensor_copy(payload[:, :D], z_sb[:, t, :])
                nc.vector.memset(payload[:, D:D + 1], 1.0)
                pm = ps_sm.tile([128, D + 1], F32, tag="sm")
                nc.tensor.matmul(out=pm[:], lhsT=M[:], rhs=payload[:],
                                 start=True, stop=True)
                merged = sp.tile([128, D + 1], F32, tag="mrg")
                nc.vector.tensor_copy(merged[:], pm[:])
                nc.gpsimd.indirect_dma_start(
                    out=ar_sums,
                    out_offset=bass.IndirectOffsetOnAxis(ap=idx_i[:], axis=0),
                    in_=merged[:], in_offset=None,
                    compute_op=ALU.add)

                # z_q gather
                nc.gpsimd.indirect_dma_start(
                    out=zq_all[:, t, :], out_offset=None, in_=emb_d,
                    in_offset=bass.IndirectOffsetOnAxis(ap=i8[:, 0:1], axis=0))

            # ---- epilogue: z_q_ste + commit partial ----
            dif = wp.tile([128, NTOK], F32, tag="E")
            zq_flat = zq_all[:].rearrange("p t d -> p (t d)")
            z_flat = z_sb[:].rearrange("p t d -> p (t d)")
            nc.vector.tensor_tensor(out=dif[:], in0=zq_flat, in1=z_flat,
                                    op=ALU.subtract)
            ste = wp.tile([128, NTOK], F32, tag="E")
            nc.vector.tensor_tensor(out=ste[:], in0=z_flat, in1=dif[:],
                                    op=ALU.add)
            nc.sync.dma_start(
                out=zq_d.rearrange("(t p) d -> p t d", p=128),
                in_=ste[:].rearrange("p (t d) -> p t d", t=NT))
            csum = sp.tile([128, 1], F32, tag="csum")
            nc.scalar.activation(out=dif[:], in_=dif[:], func=AF.Square,
                                 accum_out=csum[:])
            pc1 = ps_sm.tile([1, 1], F32, tag="sm")
            nc.tensor.matmul(out=pc1[:], lhsT=csum[:], rhs=ones_col[:],
                             start=True, stop=True)
            cstage = sp.tile([1, 1], F32, tag="cst")
            nc.scalar.activation(out=cstage[:], in_=pc1[:], func=AF.Copy,
                                 scale=BETA / (N * D))
            nc.gpsimd.dma_start(out=ar_pv[64:65, 0:1], in_=cstage[:],
                              accum_op=mybir.AluOpType.add)

            # ---- collectives ----
            nc.gpsimd.collective_compute(
                "ReduceScatter", mybir.AluOpType.add, ins=[ar_sums[0:K, :]],
                outs=[ar_sums_o[:]], replica_groups=groups)

            # ---- post-AR: EMA outputs for this core's 1024-code shard ----
            shard = cp.tile([128, 8, D + 1], F32)
            nc.sync.dma_start(
                out=shard[:],
                in_=ar_sums_o[:].rearrange("(c p) x -> p c x", p=128))
            cssb = cp.tile([128, 8], F32)
            nc.sync.dma_start(out=cssb[:],
                              in_=cs_d.rearrange("(c p) -> p c", p=128))
            emasb = cp.tile([128, 8, D], F32)
            nc.sync.dma_start(out=emasb[:],
                              in_=ema_d.rearrange("(c p) d -> p c d", p=128))

            ncs_sb = cp.tile([128, 8], F32)
            cnt_v = shard[:, :, D:D + 1].rearrange("p c o -> p (c o)")
            nc.vector.tensor_scalar(ncs_sb[:], cnt_v, 1 - DECAY, None,
                                    op0=ALU.mult)
            cs_sc = sp.tile([128, 8], F32, tag="cssc")
            nc.vector.tensor_scalar(cs_sc[:], cssb[:], DECAY, None,
                                    op0=ALU.mult)
            nc.vector.tensor_tensor(out=ncs_sb[:], in0=ncs_sb[:], in1=cs_sc[:],
                                    op=ALU.add)
            nc.sync.dma_start(out=ncs_d.rearrange("(c p) -> p c", p=128),
                              in_=ncs_sb[:])

            nema_sb = cp.tile([128, 8, D], F32)
            nc.vector.tensor_scalar(nema_sb[:], shard[:, :, :D],
                                    1 - DECAY, None, op0=ALU.mult)
            ema_sc = cp.tile([128, 8, D], F32)
            nc.vector.tensor_scalar(ema_sc[:], emasb[:],
                                    DECAY, None, op0=ALU.mult)
            nc.vector.tensor_tensor(out=nema_sb[:], in0=nema_sb[:],
                                    in1=ema_sc[:], op=ALU.add)
            nc.sync.dma_start(out=nema_d.rearrange("(c p) d -> p c d", p=128),
                              in_=nema_sb[:])

            den = sp.tile([128, 8], F32, tag="den")
            nc.vector.tensor_scalar(den[:], ncs_sb[:], 1e-5, None, op0=ALU.max)
            rden = sp.tile([128, 8], F32, tag="rden")
            nc.vector.reciprocal(rden[:], den[:])
            nemb_sb = cp.tile([128, 8, D], F32)
            for c in range(8):
                nc.vector.tensor_scalar(nemb_sb[:, c, :], nema_sb[:, c, :],
                                        rden[:, c:c + 1], None, op0=ALU.mult)
            nc.sync.dma_start(out=nemb_d.rearrange("(c p) d -> p c d", p=128),
                              in_=nemb_sb[:])

            # ---- perplexity partial over this core's shard ----
            em = sp.tile([128, 8], F32, tag="em")
            cnt_g = shard[:, :, D:D + 1].rearrange("p c o -> p (c o)")
            nc.vector.tensor_scalar(em[:], cnt_g, 1.0 / N, None, op0=ALU.mult)
            lg = sp.tile([128, 8], F32, tag="lg")
            nc.scalar.activation(out=lg[:], in_=em[:], func=AF.Ln,
                                 bias=eps8[:])
            nc.vector.tensor_tensor(out=lg[:], in0=lg[:], in1=em[:],
                                    op=ALU.mult)
            ppart = sp.tile([128, 1], F32, tag="ppart")
            nc.vector.reduce_sum(out=ppart[:], in_=lg[:], axis=AX.X)
            pp1 = ps_sm.tile([1, 1], F32, tag="sm")
            nc.tensor.matmul(out=pp1[:], lhsT=ppart[:], rhs=ones_col[:],
                             start=True, stop=True)
            pstage = sp.tile([1, 1], F32, tag="pst")
            nc.scalar.copy(out=pstage[:], in_=pp1[:])
            nc.gpsimd.dma_start(out=ar_pv[65:66, 0:1], in_=pstage[:],
                                accum_op=mybir.AluOpType.add)
            # second collective: P_avg rows + commit + perplexity partials
            nc.gpsimd.collective_compute(
                "AllReduce", mybir.AluOpType.add, ins=[ar_pv[:]],
                outs=[ar_pv_o[:]], replica_groups=groups)
            perpsum = sp.tile([1, 1], F32, tag="perp0")
            nc.sync.dma_start(out=perpsum[:], in_=ar_pv_o[65:66, 0:1])
            perp = sp.tile([1, 1], F32, tag="perp")
            nc.scalar.activation(out=perp[:], in_=perpsum[:], func=AF.Exp,
                                 scale=-1.0)

            # entropy from P_avg
            pvsb = cp.tile([64, D], F32, tag="pvsb")
            nc.sync.dma_start(out=pvsb[:], in_=ar_pv_o[0:64, :])
            Pm = sp.tile([64, D], F32, tag="Pm")
            nc.vector.tensor_scalar(Pm[:], pvsb[:], 1.0 / N, 1e-8,
                                    op0=ALU.mult, op1=ALU.add)
            lP = sp.tile([64, D], F32, tag="lP")
            nc.scalar.activation(out=lP[:], in_=Pm[:], func=AF.Ln, bias=0.0)
            nc.vector.tensor_tensor(out=lP[:], in0=lP[:], in1=Pm[:],
                                    op=ALU.mult)
            ered = sp.tile([64, 1], F32, tag="ered")
            nc.vector.reduce_sum(out=ered[:], in_=lP[:], axis=AX.X)
            pe1 = ps_sm.tile([1, 1], F32, tag="sm")
            nc.tensor.matmul(out=pe1[:], lhsT=ered[:], rhs=ones_col[:64, :],
                             start=True, stop=True)
            ent = sp.tile([1, 1], F32, tag="ent")
            nc.scalar.activation(out=ent[:], in_=pe1[:], func=AF.Copy,
                                 scale=-1.0)

            commit_sb = sp.tile([1, 1], F32, tag="comm")
            nc.sync.dma_start(out=commit_sb[:], in_=ar_pv_o[64:65, 0:1])

            scal_sb = sp.tile([1, 4], F32, tag="scal")
            nc.vector.tensor_copy(scal_sb[:, 0:1], commit_sb[:])
            nc.vector.tensor_copy(scal_sb[:, 1:2], perp[:])
            nc.vector.tensor_copy(scal_sb[:, 2:3], ent[:])
            nc.vector.memset(scal_sb[:, 3:4], 0.0)
            nc.sync.dma_start(out=scal_d[None, :], in_=scal_sb[:])

    nc.compile()
    return nc


def _get_nc():
    if "nc" not in _cached:
        _cached["nc"] = _build()
    return _cached["nc"]


def kernel(z, emb_weight, cluster_size, ema_embedding_data):
    from concourse.bass_utils import run_bass_kernel_spmd

    z = np.ascontiguousarray(z, dtype=np.float32)
    emb_weight = np.ascontiguousarray(emb_weight, dtype=np.float32)
    cluster_size = np.ascontiguousarray(cluster_size, dtype=np.float32)
    ema = np.ascontiguousarray(ema_embedding_data, dtype=np.float32)

    zf = z.reshape(N, D)
    in_maps = []
    for c in range(NCORES):
        in_maps.append({
            "z": zf[c * NTOK:(c + 1) * NTOK],
            "emb": emb_weight,
            "cs": cluster_size[c * KSH:(c + 1) * KSH],
            "ema": ema[c * KSH:(c + 1) * KSH],
        })
    nc = _get_nc()
    res = run_bass_kernel_spmd(nc, in_maps, core_ids=list(range(NCORES)))
    rs = res.results

    z_q_ste = np.concatenate([rs[c]["zq"] for c in range(NCORES)], axis=0)
    z_q_ste = z_q_ste.reshape(z.shape)
    new_cluster = np.concatenate([rs[c]["ncs"] for c in range(NCORES)])
    new_ema = np.concatenate([rs[c]["nema"] for c in range(NCORES)], axis=0)
    new_emb = np.concatenate([rs[c]["nemb"] for c in range(NCORES)], axis=0)
    scal = rs[0]["scal"]
    commit = np.float32(scal[0])
    perp = np.float32(scal[1])
    ent = np.float32(scal[2])
    return (z_q_ste, commit, perp, ent, new_cluster, new_ema, new_emb)


# revision 32
# speedup vs baseline: 1.1900x; 1.0038x over previous
"""CosVQ-EMA Trainium2 kernel (8 NeuronCores, bass/Tile).

Strategy (data-parallel over tokens, per sharding hint):
  - 16384 tokens sharded 2048/core; codebook [8192,128] replicated.
  - Per core, per 128-token tile: fp32 scores matmul (z_T stationary,
    normalized-codebook enT streamed) -> PSUM; ACT exp (1/|z| and the 1/T=10
    folded into the per-partition activation scale; softmax needs no max
    subtraction since |10*cos| <= 10) with accum_out giving row sums Z.
  - argmax via DVE max8 + max_index on E=exp(scores) (exp is monotone).
  - P_avg partials via M=1 PE matmuls (w=1/Z stationary, E streamed as
    bf16 made by an idle-GPSIMD copy) -> PSUM -> ACT copy -> DMA
    accum_op=add into a DRAM buffer.
  - counts+sums via per-tile merge matmul (duplicate-index groups summed via
    an is_equal selection matrix) + indirect DMA scatter with compute_op=add
    into a DRAM accumulator; within-tile duplicates redirected to garbage rows.
  - z_q via indirect DMA gather of emb rows.
  - One ReduceScatter over [8192,129] (sums|counts -> each core's 1024-code
    shard) + one small AllReduce over [66,128] (P_avg rows, commit and
    perplexity partials); each core then computes its EMA output shard and
    the (replicated) scalars.
Host side only shards inputs / concatenates outputs.
"""

import numpy as np

N = 16384
D = 128
K = 8192
NCORES = 8
NTOK = N // NCORES          # 2048 tokens per core
NT = NTOK // 128            # 16 tiles of 128 tokens
KSH = K // NCORES           # 1024 codes per core for EMA outputs
KC = K // 512               # 16 chunks of 512 codes
BETA = 0.25
DECAY = 0.8
INV_TEMP = 10.0
GARBAGE_ROWS = 256

_cached = {}


def _build():
    import concourse.bass as bass
    import concourse.mybir as mybir
    import concourse.tile as tile
    from concourse import bacc
    from concourse.masks import make_identity

    F32 = mybir.dt.float32
    BF16 = mybir.dt.bfloat16
    U32 = mybir.dt.uint32
    I32 = mybir.dt.int32
    AF = mybir.ActivationFunctionType
    ALU = mybir.AluOpType
    AX = mybir.AxisListType

    nc = bacc.Bacc("TRN2", target_bir_lowering=False, debug=False,
                   num_devices=NCORES)

    z_d = nc.dram_tensor("z", [NTOK, D], F32, kind="ExternalInput").ap()
    emb_d = nc.dram_tensor("emb", [K, D], F32, kind="ExternalInput").ap()
    cs_d = nc.dram_tensor("cs", [KSH], F32, kind="ExternalInput").ap()
    ema_d = nc.dram_tensor("ema", [KSH, D], F32, kind="ExternalInput").ap()

    zq_d = nc.dram_tensor("zq", [NTOK, D], F32, kind="ExternalOutput").ap()
    ncs_d = nc.dram_tensor("ncs", [KSH], F32, kind="ExternalOutput").ap()
    nema_d = nc.dram_tensor("nema", [KSH, D], F32, kind="ExternalOutput").ap()
    nemb_d = nc.dram_tensor("nemb", [KSH, D], F32, kind="ExternalOutput").ap()
    scal_d = nc.dram_tensor("scal", [4], F32, kind="ExternalOutput").ap()

    # DRAM accumulators. ar_sums rows: [0:8192] per-code [sums(128)|count];
    # rows [8192:] garbage rows absorbing within-tile duplicate scatters.
    ar_sums = nc.dram_tensor("ar_sums", [K + GARBAGE_ROWS, D + 1], F32).ap()
    ar_sums_o = nc.dram_tensor("ar_sums_o", [KSH, D + 1], F32).ap()
    # ar_pv rows [0:64] = P_avg sums as 64x128; row 64 col 0 = commit partial.
    ar_pv = nc.dram_tensor("ar_pv", [66, D], F32).ap()
    ar_pv_o = nc.dram_tensor("ar_pv_o", [66, D], F32, addr_space="Shared").ap()

    groups = [list(range(NCORES))]

    with tile.TileContext(nc) as tc:
        with tc.tile_pool(name="const", bufs=1) as cp, \
             tc.tile_pool(name="work", bufs=2) as wp, \
             tc.tile_pool(name="small", bufs=3) as sp, \
             tc.tile_pool(name="ebf", bufs=1) as ebp, \
             tc.tile_pool(name="ps_sc", bufs=2, space="PSUM") as ps_sc, \
             tc.tile_pool(name="ps_sm", bufs=2, space="PSUM") as ps_sm:

            ident = cp.tile([128, 128], F32)
            make_identity(nc, ident[:])
            ones_col = cp.tile([128, 1], F32)
            nc.vector.memset(ones_col[:], 1.0)
            zeros_row = cp.tile([128, D + 1], F32)
            nc.vector.memset(zeros_row[:], 0.0)
            iota_qp = cp.tile([128, 128], I32)
            nc.gpsimd.iota(iota_qp[:], pattern=[[1, 128]], base=0,
                           channel_multiplier=-1)
            iota_qp_f = cp.tile([128, 128], F32)
            nc.vector.tensor_copy(iota_qp_f[:], iota_qp[:])
            LT = cp.tile([128, 128], F32)
            nc.vector.tensor_scalar(LT[:], iota_qp_f[:], 0.0, None,
                                    op0=ALU.is_lt)
            gro_i = cp.tile([128, 1], I32)
            nc.gpsimd.iota(gro_i[:], pattern=[[0, 1]], base=K,
                           channel_multiplier=1)
            gro_f = cp.tile([128, 1], F32)
            nc.vector.tensor_copy(gro_f[:], gro_i[:])
            eps8 = cp.tile([128, 1], F32)
            nc.vector.memset(eps8[:], 1e-8)

            # ---- zero the DRAM accumulators ----
            arv = ar_sums.rearrange("(t p) x -> p t x", p=128)
            for t in range(arv.shape[1]):
                nc.sync.dma_start(out=arv[:, t, :], in_=zeros_row[:])
            nc.sync.dma_start(out=ar_pv, in_=zeros_row[:66, :D])

            # ---- codebook prep: enT = normalize(emb) transposed ----
            emb_sb = wp.tile([128, 64, 128], F32, tag="E")
            nc.sync.dma_start(out=emb_sb[:],
                              in_=emb_d.rearrange("(c p) d -> p c d", p=128))
            esq = wp.tile([128, 8192], F32, tag="E")
            nc.vector.tensor_tensor(out=esq[:].rearrange("p (c d) -> p c d", c=64),
                                    in0=emb_sb[:], in1=emb_sb[:], op=ALU.mult)
            en2 = cp.tile([128, 64], F32)
            nc.vector.reduce_sum(out=en2[:],
                                 in_=esq[:].rearrange("p (c d) -> p c d", c=64),
                                 axis=AX.X)
            enrm = cp.tile([128, 64], F32)
            nc.scalar.activation(out=enrm[:], in_=en2[:], func=AF.Sqrt)
            nc.vector.tensor_scalar(enrm[:], enrm[:], 1e-12, None, op0=ALU.max)
            erec = cp.tile([128, 64], F32)
            nc.vector.reciprocal(erec[:], enrm[:])
            for c in range(64):
                nc.vector.tensor_scalar(emb_sb[:, c, :], emb_sb[:, c, :],
                                        erec[:, c:c + 1], None, op0=ALU.mult)
            enTq = []
            for q in range(4):
                enT_part = cp.tile([128, 2048], F32, tag=f"enT{q}")
                enTq.append(enT_part)
            for c in range(64):
                pt = ps_sm.tile([128, 128], F32, tag="sm")
                nc.tensor.transpose(out=pt[:], in_=emb_sb[:, c, :],
                                    identity=ident[:])
                nc.scalar.copy(out=enTq[c // 16][:, (c % 16) * 128:
                                                 (c % 16 + 1) * 128],
                               in_=pt[:])

            # ---- z prep ----
            z_sb = cp.tile([128, NT, 128], F32)
            nc.sync.dma_start(out=z_sb[:],
                              in_=z_d.rearrange("(t p) d -> p t d", p=128))
            zsq = wp.tile([128, NTOK], F32, tag="E")
            nc.vector.tensor_tensor(out=zsq[:].rearrange("p (t d) -> p t d", t=NT),
                                    in0=z_sb[:], in1=z_sb[:], op=ALU.mult)
            zn2 = cp.tile([128, NT], F32)
            nc.vector.reduce_sum(out=zn2[:],
                                 in_=zsq[:].rearrange("p (t d) -> p t d", t=NT),
                                 axis=AX.X)
            znrm = cp.tile([128, NT], F32)
            nc.scalar.activation(out=znrm[:], in_=zn2[:], func=AF.Sqrt)
            nc.vector.tensor_scalar(znrm[:], znrm[:], 1e-12, None, op0=ALU.max)
            rnz10 = cp.tile([128, NT], F32)
            nc.vector.reciprocal(rnz10[:], znrm[:])
            nc.vector.tensor_scalar(rnz10[:], rnz10[:], INV_TEMP, None,
                                    op0=ALU.mult)
            zT = cp.tile([128, NT, 128], F32)
            for t in range(NT):
                pt = ps_sm.tile([128, 128], F32, tag="sm")
                nc.tensor.transpose(out=pt[:], in_=z_sb[:, t, :],
                                    identity=ident[:])
                nc.scalar.copy(out=zT[:, t, :], in_=pt[:])

            zq_all = cp.tile([128, NT, 128], F32)

            # ---- main loop over token tiles ----
            for t in range(NT):
                E = wp.tile([128, 8192], F32, tag="E")
                zparts = sp.tile([128, 8], F32, tag="zp")
                for g in range(8):
                    psc = ps_sc.tile([128, 1024], F32, tag="sc")
                    for j in range(2):
                        ch = 2 * g + j
                        nc.tensor.matmul(
                            out=psc[:, j * 512:(j + 1) * 512],
                            lhsT=zT[:, t, :],
                            rhs=enTq[ch // 4][:, (ch % 4) * 512:
                                              (ch % 4 + 1) * 512],
                            start=True, stop=True)
                    nc.scalar.activation(out=E[:, g * 1024:(g + 1) * 1024],
                                         in_=psc[:], func=AF.Exp,
                                         scale=rnz10[:, t:t + 1],
                                         accum_out=zparts[:, g:g + 1])
                Zrow = sp.tile([128, 1], F32, tag="zrow")
                nc.vector.reduce_sum(out=Zrow[:], in_=zparts[:], axis=AX.X)
                w = sp.tile([128, 1], F32, tag="w")
                nc.vector.reciprocal(w[:], Zrow[:])
                wc = sp.tile([128, 1], F32, tag="wc")
                nc.vector.tensor_scalar(wc[:], w[:], 1.00195694, None,
                                        op0=ALU.mult)
                w_bf = sp.tile([128, 1], BF16, tag="wbf")
                nc.vector.tensor_copy(w_bf[:], wc[:])
                # bf16 view of E: high half of each fp32 word (truncated bf16)
                E_bf = E[:].bitcast(BF16).rearrange("p (k two) -> p k two",
                                                    two=2)

                m32 = sp.tile([128, 32], F32, tag="m32")
                for h in range(4):
                    nc.vector.max(out=m32[:, h * 8:(h + 1) * 8],
                                  in_=E[:, h * 2048:(h + 1) * 2048])
                m8 = sp.tile([128, 8], F32, tag="m8")
                nc.vector.max(out=m8[:], in_=m32[:])
                i8 = sp.tile([128, 8], U32, tag="i8")
                nc.vector.max_index(out=i8[:], in_max=m8[:], in_values=E[:])

                # P_avg partial: [1,512] = w^T @ E chunk (bf16), staged to a
                # [1,8192] SBUF row; one DMA accumulate into ar_pv per tile.
                pvst = ebp.tile([1, 8192], F32, tag="pvst")
                for g in range(8):
                    pv = ps_sm.tile([1, 1024], F32, tag="sm")
                    for j in range(2):
                        ch = 2 * g + j
                        nc.tensor.matmul(
                            out=pv[:, j * 512:(j + 1) * 512],
                            lhsT=w_bf[:],
                            rhs=E_bf[:, ch * 512:(ch + 1) * 512, 1],
                            start=True, stop=True)
                    nc.scalar.copy(out=pvst[:, g * 1024:(g + 1) * 1024],
                                   in_=pv[:])
                nc.gpsimd.dma_start(out=ar_pv[0:64, :], in_=pvst[:],
                                    accum_op=mybir.AluOpType.add)

                # ---- merge duplicate-index rows, dedup-redirect, scatter ----
                idx_f = sp.tile([128, 1], F32, tag="idxf")
                nc.vector.tensor_copy(idx_f[:], i8[:, 0:1])
                pT = ps_sm.tile([128, 128], F32, tag="sm")
                nc.tensor.transpose(out=pT[:],
                                    in_=idx_f[:].to_broadcast([128, 128]),
                                    identity=ident[:])
                idxT = sp.tile([128, 128], F32, tag="idxT")
                nc.vector.tensor_copy(idxT[:], pT[:])
                M = sp.tile([128, 128], F32, tag="M")
                nc.vector.tensor_tensor(out=M[:],
                                        in0=idx_f[:].to_broadcast([128, 128]),
                                        in1=idxT[:], op=ALU.is_equal)
                # duplicate mask: any earlier token with same idx
                MLT = sp.tile([128, 128], F32, tag="MLT")
                nc.vector.tensor_tensor(out=MLT[:], in0=M[:], in1=LT[:],
                                        op=ALU.mult)
                dup = sp.tile([128, 1], F32, tag="dup")
                nc.vector.reduce_sum(out=dup[:], in_=MLT[:], axis=AX.X)
                dmask = sp.tile([128, 1], U32, tag="dmask")
                nc.vector.tensor_scalar(dmask[:], dup[:], 0.0, None,
                                        op0=ALU.is_gt)
                idx_sc = sp.tile([128, 1], F32, tag="idxsc")
                nc.vector.tensor_copy(idx_sc[:], idx_f[:])
                nc.vector.copy_predicated(idx_sc[:], dmask[:], gro_f[:])
                idx_i = sp.tile([128, 1], I32, tag="idxi")
                nc.vector.tensor_copy(idx_i[:], idx_sc[:])

                payload = sp.tile([128, D + 1], F32, tag="pay")
                nc.vector.tensor_copy(payload[:, :D], z_sb[:, t, :])
                nc.vector.memset(payload[:, D:D + 1], 1.0)
                pm = ps_sm.tile([128, D + 1], F32, tag="sm")
                nc.tensor.matmul(out=pm[:], lhsT=M[:], rhs=payload[:],
                                 start=True, stop=True)
                merged = sp.tile([128, D + 1], F32, tag="mrg")
                nc.vector.tensor_copy(merged[:], pm[:])
                nc.gpsimd.indirect_dma_start(
                    out=ar_sums,
                    out_offset=bass.IndirectOffsetOnAxis(ap=idx_i[:], axis=0),
                    in_=merged[:], in_offset=None,
                    compute_op=ALU.add)

                # z_q gather
                nc.gpsimd.indirect_dma_start(
                    out=zq_all[:, t, :], out_offset=None, in_=emb_d,
                    in_offset=bass.IndirectOffsetOnAxis(ap=i8[:, 0:1], axis=0))

            # ---- epilogue: z_q_ste + commit partial ----
            dif = wp.tile([128, NTOK], F32, tag="E")
            zq_flat = zq_all[:].rearrange("p t d -> p (t d)")
            z_flat = z_sb[:].rearrange("p t d -> p (t d)")
            nc.vector.tensor_tensor(out=dif[:], in0=zq_flat, in1=z_flat,
                                    op=ALU.subtract)
            ste = wp.tile([128, NTOK], F32, tag="E")
            nc.vector.tensor_tensor(out=ste[:], in0=z_flat, in1=dif[:],
                                    op=ALU.add)
            nc.sync.dma_start(
                out=zq_d.rearrange("(t p) d -> p t d", p=128),
                in_=ste[:].rearrange("p (t d) -> p t d", t=NT))
            csum = sp.tile([128, 1], F32, tag="csum")
            nc.scalar.activation(out=dif[:], in_=dif[:], func=AF.Square,
                                 accum_out=csum[:])
            pc1 = ps_sm.tile([1, 1], F32, tag="sm")
            nc.tensor.matmul(out=pc1[:], lhsT=csum[:], rhs=ones_col[:],
                             start=True, stop=True)
            cstage = sp.tile([1, 1], F32, tag="cst")
            nc.scalar.activation(out=cstage[:], in_=pc1[:], func=AF.Copy,
                                 scale=BETA / (N * D))
            nc.gpsimd.dma_start(out=ar_pv[64:65, 0:1], in_=cstage[:],
                              accum_op=mybir.AluOpType.add)

            # ---- collectives ----
            nc.gpsimd.collective_compute(
                "ReduceScatter", mybir.AluOpType.add, ins=[ar_sums[0:K, :]],
                outs=[ar_sums_o[:]], replica_groups=groups)

            # ---- post-AR: EMA outputs for this core's 1024-code shard ----
            shard = cp.tile([128, 8, D + 1], F32)
            nc.sync.dma_start(
                out=shard[:],
                in_=ar_sums_o[:].rearrange("(c p) x -> p c x", p=128))
            cssb = cp.tile([128, 8], F32)
            nc.sync.dma_start(out=cssb[:],
                              in_=cs_d.rearrange("(c p) -> p c", p=128))
            emasb = cp.tile([128, 8, D], F32)
            nc.sync.dma_start(out=emasb[:],
                              in_=ema_d.rearrange("(c p) d -> p c d", p=128))

            ncs_sb = cp.tile([128, 8], F32)
            cnt_v = shard[:, :, D:D + 1].rearrange("p c o -> p (c o)")
            nc.vector.tensor_scalar(ncs_sb[:], cnt_v, 1 - DECAY, None,
                                    op0=ALU.mult)
            cs_sc = sp.tile([128, 8], F32, tag="cssc")
            nc.vector.tensor_scalar(cs_sc[:], cssb[:], DECAY, None,
                                    op0=ALU.mult)
            nc.vector.tensor_tensor(out=ncs_sb[:], in0=ncs_sb[:], in1=cs_sc[:],
                                    op=ALU.add)
            nc.sync.dma_start(out=ncs_d.rearrange("(c p) -> p c", p=128),
                              in_=ncs_sb[:])

            nema_sb = cp.tile([128, 8, D], F32)
            nc.vector.tensor_scalar(nema_sb[:], shard[:, :, :D],
                                    1 - DECAY, None, op0=ALU.mult)
            ema_sc = cp.tile([128, 8, D], F32)
            nc.vector.tensor_scalar(ema_sc[:], emasb[:],
                                    DECAY, None, op0=ALU.mult)
            nc.vector.tensor_tensor(out=nema_sb[:], in0=nema_sb[:],
                                    in1=ema_sc[:], op=ALU.add)
            nc.sync.dma_start(out=nema_d.rearrange("(c p) d -> p c d", p=128),
                              in_=nema_sb[:])

            den = sp.tile([128, 8], F32, tag="den")
            nc.vector.tensor_scalar(den[:], ncs_sb[:], 1e-5, None, op0=ALU.max)
            rden = sp.tile([128, 8], F32, tag="rden")
            nc.vector.reciprocal(rden[:], den[:])
            nemb_sb = cp.tile([128, 8, D], F32)
            for c in range(8):
                nc.vector.tensor_scalar(nemb_sb[:, c, :], nema_sb[:, c, :],
                                        rden[:, c:c + 1], None, op0=ALU.mult)
            nc.sync.dma_start(out=nemb_d.rearrange("(c p) d -> p c d", p=128),
                              in_=nemb_sb[:])

            # ---- perplexity partial over this core's shard ----
            em = sp.tile([128, 8], F32, tag="em")
            cnt_g = shard[:, :, D:D + 1].rearrange("p c o -> p (c o)")
            nc.vector.tensor_scalar(em[:], cnt_g, 1.0 / N, None, op0=ALU.mult)
            lg = sp.tile([128, 8], F32, tag="lg")
            nc.scalar.activation(out=lg[:], in_=em[:], func=AF.Ln,
                                 bias=eps8[:])
            nc.vector.tensor_tensor(out=lg[:], in0=lg[:], in1=em[:],
                                    op=ALU.mult)
            ppart = sp.tile([128, 1], F32, tag="ppart")
            nc.vector.reduce_sum(out=ppart[:], in_=lg[:], axis=AX.X)
            pp1 = ps_sm.tile([1, 1], F32, tag="sm")
            nc.tensor.matmul(out=pp1[:], lhsT=ppart[:], rhs=ones_col[:],
                             start=True, stop=True)
            pstage = sp.tile([1, 1], F32, tag="pst")
            nc.scalar.copy(out=pstage[:], in_=pp1[:])
            nc.gpsimd.dma_start(out=ar_pv[65:66, 0:1], in_=pstage[:],
                                accum_op=mybir.AluOpType.add)
            # second collective: P_avg rows + commit + perplexity partials
            nc.gpsimd.collective_compute(
                "AllReduce", mybir.AluOpType.add, ins=[ar_pv[:]],
                outs=[ar_pv_o[:]], replica_groups=groups)
            perpsum = sp.tile([1, 1], F32, tag="perp0")
            nc.sync.dma_start(out=perpsum[:], in_=ar_pv_o[65:66, 0:1])
            perp = sp.tile([1, 1], F32, tag="perp")
            nc.scalar.activation(out=perp[:], in_=perpsum[:], func=AF.Exp,
                                 scale=-1.0)

            # entropy from P_avg
            pvsb = cp.tile([64, D], F32, tag="pvsb")
            nc.sync.dma_start(out=pvsb[:], in_=ar_pv_o[0:64, :])
            Pm = sp.tile([64, D], F32, tag="Pm")
            nc.vector.tensor_scalar(Pm[:], pvsb[:], 1.0 / N, 1e-8,
                                    op0=ALU.mult, op1=ALU.add)
            lP = sp.tile([64, D], F32, tag="lP")
            nc.scalar.activation(out=lP[:], in_=Pm[:], func=AF.Ln, bias=0.0)
            nc.vector.tensor_tensor(out=lP[:], in0=lP[:], in1=Pm[:],
                                    op=ALU.mult)
            ered = sp.tile([64, 1], F32, tag="ered")
            nc.vector.reduce_sum(out=ered[:], in_=lP[:], axis=AX.X)
            pe1 = ps_sm.tile([1, 1], F32, tag="sm")
            nc.tensor.matmul(out=pe1[:], lhsT=ered[:], rhs=ones_col[:64, :],
                             start=True, stop=True)
            ent = sp.tile([1, 1], F32, tag="ent")
            nc.scalar.activation(out=ent[:], in_=pe1[:], func=AF.Copy,
                                 scale=-1.0)

            commit_sb = sp.tile([1, 1], F32, tag="comm")
            nc.sync.dma_start(out=commit_sb[:], in_=ar_pv_o[64:65, 0:1])

            scal_sb = sp.tile([1, 4], F32, tag="scal")
            nc.vector.tensor_copy(scal_sb[:, 0:1], commit_sb[:])
            nc.vector.tensor_copy(scal_sb[:, 1:2], perp[:])
            nc.vector.tensor_copy(scal_sb[:, 2:3], ent[:])
            nc.vector.memset(scal_sb[:, 3:4], 0.0)
            nc.sync.dma_start(out=scal_d[None, :], in_=scal_sb[:])

    nc.compile()
    return nc


def _get_nc():
    if "nc" not in _cached:
        _cached["nc"] = _build()
    return _cached["nc"]


def kernel(z, emb_weight, cluster_size, ema_embedding_data):
    from concourse.bass_utils import run_bass_kernel_spmd

    z = np.ascontiguousarray(z, dtype=np.float32)
    emb_weight = np.ascontiguousarray(emb_weight, dtype=np.float32)
    cluster_size = np.ascontiguousarray(cluster_size, dtype=np.float32)
    ema = np.ascontiguousarray(ema_embedding_data, dtype=np.float32)

    zf = z.reshape(N, D)
    in_maps = []
    for c in range(NCORES):
        in_maps.append({
            "z": zf[c * NTOK:(c + 1) * NTOK],
            "emb": emb_weight,
            "cs": cluster_size[c * KSH:(c + 1) * KSH],
            "ema": ema[c * KSH:(c + 1) * KSH],
        })
    nc = _get_nc()
    res = run_bass_kernel_spmd(nc, in_maps, core_ids=list(range(NCORES)))
    rs = res.results

    z_q_ste = np.concatenate([rs[c]["zq"] for c in range(NCORES)], axis=0)
    z_q_ste = z_q_ste.reshape(z.shape)
    new_cluster = np.concatenate([rs[c]["ncs"] for c in range(NCORES)])
    new_ema = np.concatenate([rs[c]["nema"] for c in range(NCORES)], axis=0)
    new_emb = np.concatenate([rs[c]["nemb"] for c in range(NCORES)], axis=0)
    scal = rs[0]["scal"]
    commit = np.float32(scal[0])
    perp = np.float32(scal[1])
    ent = np.float32(scal[2])
    return (z_q_ste, commit, perp, ent, new_cluster, new_ema, new_emb)
